# revision 1
# baseline (speedup 1.0000x reference)
"""MoE FFN (grouped top-1 routing, SwiGLU experts) on 8 Trainium2 NeuronCores.

Strategy (expert-parallel with quarter-FFN sharding for perfect balance):
  - Host computes the (tiny) routers: sigmoid(x @ macro_w) -> top-1 group of 4;
    within the selected group both 2 experts are active with
    sigmoid-normalized weights.
  - Tokens are sorted by routed group into one replicated array xs[D, W]
    (per-group segments at fixed padded offsets).
  - The 8 experts x 4 F-quarters = 32 weight shards are dealt so that every
    core gets exactly one shard of each GROUP (core c, group g -> expert
    2g + c//4, F-quarter c%4).  Every core therefore runs the identical
    amount of work on identically-shaped segments: perfect SPMD balance.
  - Device: for each group segment, Y_q^T = dwq^T @ (silu(gwq^T X^T) *
    (uwq^T X^T)) with features on partitions, tokens on the free dim, bf16
    in / fp32 PSUM / fp16 partial outputs.
  - Host combines: per token, y = w0 * sum(4 quarter partials of expert A)
    + w1 * sum(quarter partials of expert B), then unsorts.  The per-token
    router weights are applied host-side (linear in the down-projection),
    so no weighted copy of x needs to be shipped.
"""


import ml_dtypes
import numpy as np

import concourse.bass as bass  # noqa: F401  (bass types via bacc)
import concourse.mybir as mybir
import concourse.tile as tile
from concourse import bacc
from concourse.bass_utils import run_bass_kernel_spmd

P = 128
D_MODEL = 1024
FFN_DIM = 2048
NUM_EXPERTS = 8
NUM_GROUPS = 4
FQ = FFN_DIM // 4  # F-quarter = 512
DO = D_MODEL // P  # 8 k-tiles over D
FO = FQ // P  # 4 f-tiles over an F-quarter
EPS = 1e-9

F32 = mybir.dt.float32
F16 = mybir.dt.float16
BF16 = mybir.dt.bfloat16

N_CORES = 8
N_WARM = 94  # dummy matmuls to lift the PE HAM throttle during DMA startup

_BUILD_CACHE: dict[tuple, object] = {}
LAST_RESULTS = None  # stashed BassKernelResults for test harnesses


def _build(caps: tuple[tuple[int, int], ...]):
    """Bass/Tile program: 4 group segments, each one (expert, F-quarter) shard.

    caps: per group (chunk, nch); segment capacity C_g = chunk*nch.
    """
    Cs = [ch * nc_ for ch, nc_ in caps]
    offs = [sum(Cs[:g]) for g in range(NUM_GROUPS)]
    W = sum(Cs)

    nc = bacc.Bacc(
        "TRN2",
        target_bir_lowering=False,
        debug=False,
        enable_asserts=False,
        num_devices=N_CORES,
    )
    ch0 = caps[0][0]
    C0 = Cs[0]
    m1 = offs[2]
    # All inputs ship in partition-major layout matching the SBUF tiles
    # exactly: per-partition rows are fully contiguous -> max DMA bursts.
    xrests = {}
    for nm, a, b in (("xb2", ch0, C0), ("xg1", C0, m1), ("xg23", m1, W)):
        if b > a:
            xrests[nm, a, b] = nc.dram_tensor(
                nm, [P, DO, b - a], BF16, kind="ExternalInput"
            ).ap()
    # boot packs: tokens+fo0 weights / fo1 weights / fo2+3 weights — each
    # one dma_start (descriptor issue costs ~0.7us of queue time apiece)
    boot1 = nc.dram_tensor(
        "boot1", [P, DO, ch0 + 2 * P], BF16, kind="ExternalInput"
    ).ap()
    boot2 = nc.dram_tensor("boot2", [P, DO, 2 * P], BF16, kind="ExternalInput").ap()
    boot3 = nc.dram_tensor("boot3", [P, DO, 4 * P], BF16, kind="ExternalInput").ap()
    bgus = {
        g: nc.dram_tensor(f"bgu{g}", [P, DO, 2 * FQ], BF16, kind="ExternalInput").ap()
        for g in range(1, NUM_GROUPS)
    }
    bdws = [
        nc.dram_tensor(f"bdw{g}", [P, FO, D_MODEL], BF16, kind="ExternalInput").ap()
        for g in range(NUM_GROUPS)
    ]
    yt = nc.dram_tensor("yt", [D_MODEL, W], F16, kind="ExternalOutput").ap()
    wy = nc.dram_tensor("wy", [P, 64], F32, kind="ExternalOutput").ap()

    ytr = yt.rearrange("(do p) c -> p do c", p=P)

    with tile.TileContext(nc) as tc:
        with (
            tc.tile_pool(name="wu", bufs=1) as wup,
            tc.tile_pool(name="xp", bufs=1) as xp,
            tc.tile_pool(name="hp", bufs=1) as hp,
            tc.tile_pool(name="gp", bufs=1) as gp,
            tc.tile_pool(name="dp", bufs=1) as dp,
            tc.tile_pool(name="sp", bufs=4) as sp,
            tc.tile_pool(name="yp", bufs=10) as yp,
            tc.tile_pool(name="pg", bufs=2, space="PSUM") as pgp,
            tc.tile_pool(name="pu", bufs=2, space="PSUM") as pup,
            tc.tile_pool(name="pd", bufs=4, space="PSUM") as pdp,
        ):
            # ── PE warm-up: dense dummy matmuls while the startup DMAs fly ──
            wt = wup.tile([P, P], BF16, tag="wt")
            nc.vector.memset(wt[:], 0.0)
            pws = pgp.tile([P, 64], F32, tag="psg", name="psg_warm")
            for i in range(N_WARM):
                nc.tensor.matmul(
                    pws[:], wt[:], wt[:, 0:64],
                    start=(i == 0), stop=(i == N_WARM - 1),
                )

            # ── all input DMAs on ONE queue (sync), in exact need-order ─────
            # HBM bandwidth (~360 GB/s/core) is shared across queues, so a
            # second concurrent input stream only delays the critical path.
            # dedicated contiguous boot tile for the first chunk's tokens
            # (contiguous src AND dst -> ~5 KB packets instead of 600 B)
            bt1 = xp.tile([P, DO, ch0 + 2 * P], BF16, tag="bt1")
            nc.sync.dma_start(bt1[:], boot1[:])
            xss = xp.tile([P, DO, W], BF16, tag="xs")
            bt2 = gp.tile([P, DO, 2 * P], BF16, tag="bt2")
            nc.sync.dma_start(bt2[:], boot2[:])
            bt3 = gp.tile([P, DO, 4 * P], BF16, tag="bt3")
            nc.sync.dma_start(bt3[:], boot3[:])
            for (nm, a, b), src in xrests.items():
                if nm != "xb2":
                    continue
                nc.sync.dma_start(xss[:, :, a:b], src[:])
            guts = {}
            guts[1] = gp.tile([P, DO, 2 * FQ], BF16, tag="gut1", name="gut_1")
            nc.sync.dma_start(guts[1][:], bgus[1][:])
            for (nm, a, b), src in xrests.items():
                if nm != "xg1":
                    continue
                nc.sync.dma_start(xss[:, :, a:b], src[:])
            guts[2] = gp.tile([P, DO, 2 * FQ], BF16, tag="gut2", name="gut_2")
            nc.sync.dma_start(guts[2][:], bgus[2][:])
            for (nm, a, b), src in xrests.items():
                if nm != "xg23":
                    continue
                nc.sync.dma_start(xss[:, :, a:b], src[:])
            guts[3] = gp.tile([P, DO, 2 * FQ], BF16, tag="gut3", name="gut_3")
            nc.sync.dma_start(guts[3][:], bgus[3][:])
            dts = {}
            for g in range(NUM_GROUPS):
                dts[g] = dp.tile([P, FO, D_MODEL], BF16, tag=f"dt{g}", name=f"dt_{g}")
                nc.sync.dma_start(dts[g][:], bdws[g][:])

            # keep the warm-up matmuls from being dead-code-eliminated
            wys = wup.tile([P, 64], F32, tag="wy")
            nc.vector.tensor_copy(out=wys[:], in_=pws[:])
            nc.sync.dma_start(wy[:, :], wys[:])

            # ── phase 1: gate/up + SwiGLU for all 4 group segments ──────────
            hs = {}
            for g in range(NUM_GROUPS):
                chunk, nch = caps[g]
                hs[g] = hp.tile([P, FO, Cs[g]], BF16, tag=f"h{g}", name=f"h{g}")
                for cc in range(nch):
                    cs = slice(cc * chunk, (cc + 1) * chunk)
                    xcs = slice(offs[g] + cc * chunk, offs[g] + (cc + 1) * chunk)
                    for fo in range(FO):
                        if g == 0:
                            gt_, go_, ut_, uo_ = (
                                (bt1, ch0, bt1, ch0 + P),
                                (bt2, 0, bt2, P),
                                (bt3, 0, bt3, 2 * P),
                                (bt3, P, bt3, 3 * P),
                            )[fo]
                        else:
                            gt_, go_ = guts[g], fo * P
                            ut_, uo_ = guts[g], FQ + fo * P
                        psg = pgp.tile([P, chunk], F32, tag="psg",
                                       name=f"psg_{g}_{cc}_{fo}")
                        psu = pup.tile([P, chunk], F32, tag="psu",
                                       name=f"psu_{g}_{cc}_{fo}")
                        boot = g == 0 and cc == 0
                        for do in range(DO):
                            nc.tensor.matmul(
                                psg[:], gt_[:, do, go_ : go_ + P],
                                bt1[:, do, 0:ch0] if boot else xss[:, do, xcs],
                                start=(do == 0), stop=(do == DO - 1),
                            )
                        for do in range(DO):
                            nc.tensor.matmul(
                                psu[:], ut_[:, do, uo_ : uo_ + P],
                                bt1[:, do, 0:ch0] if boot else xss[:, do, xcs],
                                start=(do == 0), stop=(do == DO - 1),
                            )
                        sg = sp.tile([P, chunk], F32, tag="sg")
                        nc.scalar.activation(
                            sg[:], psg[:], mybir.ActivationFunctionType.Silu
                        )
                        nc.vector.tensor_mul(
                            out=hs[g][:, fo, cs], in0=sg[:], in1=psu[:]
                        )

            # ── phase 2: down-projection for all 4 group segments ───────────
            nq = 0
            for g in range(NUM_GROUPS):
                chunk, nch = caps[g]
                for cc in range(nch):
                    cs = slice(cc * chunk, (cc + 1) * chunk)
                    xcs = slice(offs[g] + cc * chunk, offs[g] + (cc + 1) * chunk)
                    for do in range(DO):
                        psy = pdp.tile([P, chunk], F32, tag="psy",
                                       name=f"psy_{g}_{cc}_{do}")
                        for fo in range(FO):
                            nc.tensor.matmul(
                                psy[:],
                                dts[g][:, fo, do * P : (do + 1) * P],
                                hs[g][:, fo, cs],
                                start=(fo == 0), stop=(fo == FO - 1),
                            )
                        yo = yp.tile([P, chunk], F16, tag="yo")
                        if nq % 2 == 0:
                            nc.scalar.activation(
                                yo[:], psy[:], mybir.ActivationFunctionType.Copy
                            )
                        else:
                            nc.vector.tensor_copy(out=yo[:], in_=psy[:])
                        if nq % 2 == 0:
                            nc.sync.dma_start(ytr[:, do, xcs], yo[:])
                        else:
                            nc.scalar.dma_start(ytr[:, do, xcs], yo[:])
                        nq += 1
    nc.finalize()
    return nc


def _get_program(caps: tuple[tuple[int, int], ...]):
    if caps not in _BUILD_CACHE:
        _BUILD_CACHE[caps] = _build(caps)
    return _BUILD_CACHE[caps]


def _sigmoid(z):
    return 1.0 / (1.0 + np.exp(-z))


def _route(xf32, macro_w, micro_w):
    """Host routers in float64. Returns group index per token and per-token
    weights for the 2 experts of the selected group (float32)."""
    xf = xf32.astype(np.float64)
    ms = _sigmoid(xf @ macro_w.astype(np.float64))  # [T, G]
    g_sel = np.argmax(ms, axis=1)
    T = xf.shape[0]
    mval = ms[np.arange(T), g_sel]
    mv = mval / (mval + EPS)

    w2 = np.zeros((T, 2), np.float64)
    for g in range(NUM_GROUPS):
        idx = np.nonzero(g_sel == g)[0]
        if idx.size == 0:
            continue
        s = _sigmoid(xf[idx] @ micro_w[g].astype(np.float64))  # [n, 2]
        denom = np.maximum(s[:, 0], s[:, 1]) + np.minimum(s[:, 0], s[:, 1]) + EPS
        w2[idx, 0] = mv[idx] * s[:, 0] / denom
        w2[idx, 1] = mv[idx] * s[:, 1] / denom
    return g_sel, w2.astype(np.float32)


def _cap(n: int):
    """Segment capacity: (chunk, nch) with chunk*nch >= n, chunk <= 512, %4."""
    n = max(n, 8)
    nch = -(-n // 512)
    chunk = -(-(-(-n // nch)) // 4) * 4
    return chunk, nch


def kernel(x, macro_w, micro_w, gate_w, up_w, down_w):
    global LAST_RESULTS
    x = np.asarray(x)
    B, S, D = x.shape
    T = B * S
    xf = np.ascontiguousarray(x.reshape(T, D).astype(np.float32, copy=False))

    g_sel, w2 = _route(xf, np.asarray(macro_w), np.asarray(micro_w))
    idx_by_g = [np.nonzero(g_sel == g)[0] for g in range(NUM_GROUPS)]

    caps = tuple(_cap(ix.size) for ix in idx_by_g)
    Cs = [ch * nc_ for ch, nc_ in caps]
    offs = [sum(Cs[:g]) for g in range(NUM_GROUPS)]
    W = sum(Cs)
    nc = _get_program(caps)

    # group-sorted, padded token matrix [D, W] bf16 (replicated to all cores)
    xs = np.zeros((D, W), ml_dtypes.bfloat16)
    for g in range(NUM_GROUPS):
        ix = idx_by_g[g]
        if ix.size:
            xs[:, offs[g] : offs[g] + ix.size] = xf[ix].T.astype(ml_dtypes.bfloat16)

    # bf16 weights in partition-major [p, do/fo, f/d] layout (contiguous DMA)
    gate_b = np.asarray(gate_w, np.float32).astype(ml_dtypes.bfloat16)
    up_b = np.asarray(up_w, np.float32).astype(ml_dtypes.bfloat16)
    down_b = np.asarray(down_w, np.float32).astype(ml_dtypes.bfloat16)
    # [E, D, F] -> [E, DO, P, F] -> [E, P, DO, F]
    gate_p = gate_b.reshape(NUM_EXPERTS, DO, P, FFN_DIM).transpose(0, 2, 1, 3)
    up_p = up_b.reshape(NUM_EXPERTS, DO, P, FFN_DIM).transpose(0, 2, 1, 3)
    # [E, F, D] -> [E, 4, FO, P, D] -> [E, 4, P, FO, D]
    down_p = down_b.reshape(NUM_EXPERTS, 4, FO, P, D_MODEL).transpose(0, 1, 3, 2, 4)

    # partition-major token array [p, do, c]; ship as contiguous blocks
    xsp = xs.reshape(DO, P, W).transpose(1, 0, 2)
    ch0 = caps[0][0]
    C0 = Cs[0]
    m1 = offs[2]
    xboot = np.ascontiguousarray(xsp[:, :, 0:ch0])
    xparts = {}
    for nm, a, b in (("xb2", ch0, C0), ("xg1", C0, m1), ("xg23", m1, W)):
        if b > a:
            xparts[nm] = np.ascontiguousarray(xsp[:, :, a:b])

    in_maps = []
    for c in range(N_CORES):
        m = dict(xparts)
        b = c // 4  # which expert of each group
        q = c % 4  # which F-quarter
        fsl = slice(q * FQ, (q + 1) * FQ)
        for g in range(NUM_GROUPS):
            e = 2 * g + b
            if g == 0:
                gq = gate_p[e][:, :, fsl]
                uq = up_p[e][:, :, fsl]
                m["boot1"] = np.ascontiguousarray(
                    np.concatenate(
                        [xboot, gq[:, :, 0:P], uq[:, :, 0:P]], axis=2
                    )
                )
                m["boot2"] = np.ascontiguousarray(
                    np.concatenate(
                        [gq[:, :, P : 2 * P], uq[:, :, P : 2 * P]], axis=2
                    )
                )
                m["boot3"] = np.ascontiguousarray(
                    np.concatenate(
                        [gq[:, :, 2 * P :], uq[:, :, 2 * P :]], axis=2
                    )
                )
            else:
                bgu = np.empty((P, DO, 2 * FQ), ml_dtypes.bfloat16)
                bgu[:, :, :FQ] = gate_p[e][:, :, fsl]
                bgu[:, :, FQ:] = up_p[e][:, :, fsl]
                m[f"bgu{g}"] = bgu
            m[f"bdw{g}"] = np.ascontiguousarray(down_p[e, q])
        in_maps.append(m)

    res = run_bass_kernel_spmd(nc, in_maps, core_ids=list(range(N_CORES)))
    LAST_RESULTS = res

    y = np.zeros((T, D), np.float32)
    for g in range(NUM_GROUPS):
        ix = idx_by_g[g]
        if ix.size == 0:
            continue
        seg = slice(offs[g], offs[g] + ix.size)
        pa = np.zeros((D, ix.size), np.float32)
        pb = np.zeros((D, ix.size), np.float32)
        for c in range(4):
            pa += res.results[c]["yt"][:, seg]
        for c in range(4, 8):
            pb += res.results[c]["yt"][:, seg]
        y[ix] = pa.T * w2[ix, 0:1] + pb.T * w2[ix, 1:2]
    return y.reshape(B, S, D)

